# revision 12
# baseline (speedup 1.0000x reference)
"""FCOS head (nn_FCOS_73787538145418) Trainium2 Bass kernel.

Sharding: data-parallel, one image per NeuronCore (B=8 across 8 cores),
weights replicated. Each core runs the identical SPMD NEFF over its image.

Per level (p3 64x64, p4 32x32, p5 16x16): two 4-layer 3x3 conv stems
(cls/box, 256ch + ReLU), then prediction convs (cls 20ch; box+ctr 5ch).
All PE work is bfloat16 (fp32 PSUM): bf16 weight loads get FWL so
LDWEIGHTS hides fully under the rhs stream.

p3/p4 stems use Winograd F(2,3) along H: per output-row pair q with
input rows d0..d3, the four planes D0=d0-d2, D1=d1+d2, D2=d2-d1,
D3=d1-d3 convolved (direct 3-tap along W, channel-contracted on the PE)
with ky-transformed weights G0=g0, G1=(g0+g1+g2)/2, G2=(g0-g1+g2)/2,
G3=g2 give even rows m1+m2+m3 and odd rows m2-m3-m4 -- 24 matmuls per
512-pixel tile instead of 36 (1.5x less PE time). The 4 m-planes live in
4 PSUM banks; DVE+scalar combine them (one PSUM operand per DVE op) and
relu+bias lands rows back to a spatial buffer, which DVE re-transforms
into D planes for the next layer (stride-2-row reads, packed bf16 2x).
Level inputs ship from the host already in D-plane form; the box stem
re-fetches p3's feat D planes mid-kernel (2 D buffers ping-pong). p5
(16x16) stays a direct 18-matmul conv. Prediction convs run 4-way
column-tiled on the PE (cls ci0/ci1, box ci0/ci1 in separate 32-column
strips, concurrent streams), halves summed on DVE. Output is [25, 5376]
channel-major per core; the host transposes and stacks to (8, 5376, 25).
"""
import sys

if '/opt/trn_rl_repo' not in sys.path:
    sys.path.insert(0, '/opt/trn_rl_repo')

import numpy as np
import ml_dtypes

import concourse.mybir as mybir
from concourse import bacc
import concourse.tile as tile
from concourse.bass_utils import run_bass_kernel_spmd

P = 128
NCH = 2                 # 256 channels = 2 chunks of 128
NL = 4                  # stem depth
NPIX_TOTAL = 5376
BF16 = mybir.dt.bfloat16
F32 = mybir.dt.float32
ADD = mybir.AluOpType.add
SUB = mybir.AluOpType.subtract
RELU = mybir.ActivationFunctionType.Relu
IDENT = mybir.ActivationFunctionType.Identity

_cached = {}
_run_opts = {}   # extra kwargs for run_bass_kernel_spmd (test harness: trace)
_last = {}       # last BassKernelResults (test harness reads exec_time_ns)


def _pad_view(flat_tile, off, H, W):
    n = NCH * (H + 2) * (W + 2)
    return flat_tile[:, off:off + n].rearrange(
        "p (c h w) -> p c h w", c=NCH, h=H + 2, w=W + 2)


def _d_view(flat_tile, off, Q, Wp):
    n = NCH * 4 * Q * Wp
    return flat_tile[:, off:off + n].rearrange(
        "p (c q j w) -> p c q j w", c=NCH, q=Q, j=4, w=Wp)


def _zero_ring(nc, v, H, W):
    for c in range(NCH):
        nc.vector.memset(v[:, c, 0, :], 0.0)
        nc.vector.memset(v[:, c, H + 1, :], 0.0)
        nc.vector.memset(v[:, c, 1:H + 1, 0], 0.0)
        nc.vector.memset(v[:, c, 1:H + 1, W + 1], 0.0)


def _transform(nc, spat, dstD, c, q0, q1):
    """Spatial (padded, ring-zeroed) chunk c, row-pair band [q0, q1) ->
    4 Winograd D planes on DVE."""
    sv = spat[:, c]
    n = q1 - q0

    def rows(start):
        return sv[:, start:start + 2 * n - 1:2, :]

    d0 = rows(2 * q0)
    d1 = rows(2 * q0 + 1)
    d2 = rows(2 * q0 + 2)
    d3 = rows(2 * q0 + 3)
    D = dstD[:, c, q0:q1]
    nc.vector.tensor_tensor(D[:, :, 0], d0, d2, SUB)
    nc.vector.tensor_tensor(D[:, :, 1], d1, d2, ADD)
    nc.vector.tensor_tensor(D[:, :, 2], d2, d1, SUB)
    nc.vector.tensor_tensor(D[:, :, 3], d1, d3, SUB)


def _wino_layer(nc, psum_pool, st_pool, twt, srcD, dstS, bias_ap,
                Q, W, RQ, tag, xform_to=None):
    """Winograd-H 3x3 conv 256->256 + bias + relu.

    srcD: [P, c, j, Q, W+2] D planes; dstS: padded spatial out view.
    After each o-chunk (= next layer's ci-chunk) completes, optionally
    forward-transform it into xform_to so the next layer overlaps."""
    n_tiles = Q // RQ
    for o in range(NCH):
        for it in range(n_tiles):
            q0 = it * RQ
            ps = psum_pool.tile([P, 4, RQ, W], F32, tag="ps",
                                name=f"wp_{tag}_{o}_{it}")
            for c in range(NCH):
                for j in range(4):
                    for kx in range(3):
                        nc.tensor.matmul(
                            ps[:, j], twt[:, c, o, kx, j],
                            srcD[:, c, q0:q0 + RQ, j, kx:kx + W],
                            start=(c == 0 and kx == 0),
                            stop=(c == 1 and kx == 2))
            u = st_pool.tile([P, RQ, W], BF16, tag="wu", bufs=2,
                             name=f"wu_{tag}_{o}_{it}")
            t1 = st_pool.tile([P, RQ, W], BF16, tag="wt1", bufs=2,
                              name=f"t1_{tag}_{o}_{it}")
            t2 = st_pool.tile([P, RQ, W], BF16, tag="wt2", bufs=2,
                              name=f"t2_{tag}_{o}_{it}")
            tmp = st_pool.tile([P, RQ, 2, W], BF16, tag="wtm", bufs=2,
                               name=f"tm_{tag}_{o}_{it}")
            # even rows = m1+m2+m3, odd rows = m2-m3-m4; DVE takes at most
            # one PSUM operand, so scalar first lands m2 in SBUF.
            nc.scalar.activation(u[:], ps[:, 1], IDENT)
            nc.vector.tensor_tensor(t1[:], u[:], ps[:, 0], ADD)
            nc.vector.tensor_tensor(t2[:], u[:], ps[:, 2], SUB)
            nc.vector.tensor_tensor(tmp[:, :, 0, :], t1[:], ps[:, 2], ADD)
            nc.vector.tensor_tensor(tmp[:, :, 1, :], t2[:], ps[:, 3], SUB)
            r0 = 2 * q0 + 1
            nc.scalar.activation(dstS[:, o, r0:r0 + 2 * RQ, 1:W + 1],
                                 tmp.rearrange("p q t w -> p (q t) w"),
                                 RELU, bias=bias_ap[:, o])
            if xform_to is not None and n_tiles > 1 and it > 0:
                # transform the q band whose input rows are now complete
                # (pair q needs rows 2q..2q+3; after unit it rows
                # 1..RQ*2*(it+1) exist): [0,15) / [15,23) / [23,32) at RQ=8
                b0 = 0 if it == 1 else RQ * it - 1
                b1 = Q if it == n_tiles - 1 else RQ * (it + 1) - 1
                _transform(nc, dstS, xform_to, o, b0, b1)
        if xform_to is not None and n_tiles == 1:
            _transform(nc, dstS, xform_to, o, 0, Q)


def _conv_layer(nc, psum_pool, wt, src, dst, bias_ap, H, W, R, tag):
    """Direct 3x3 same conv 256->256 + bias + relu (p5)."""
    n_tiles = H // R
    for o in range(NCH):
        pss = [
            psum_pool.tile([P, R, W], F32, tag="ps", name=f"ps_{tag}_{o}_{it}")
            for it in range(n_tiles)
        ]
        k = 0
        for c in range(NCH):
            for ky in range(3):
                for kx in range(3):
                    lhsT = wt[:, c, o, ky * 3 + kx, :]
                    for it in range(n_tiles):
                        r0 = it * R
                        rhs = src[:, c, r0 + ky:r0 + ky + R, kx:kx + W]
                        nc.tensor.matmul(pss[it][:], lhsT, rhs,
                                         start=(k == 0), stop=(k == 17))
                    k += 1
        for it in range(n_tiles):
            r0 = it * R
            nc.scalar.activation(dst[:, o, r0 + 1:r0 + 1 + R, 1:W + 1],
                                 pss[it][:], RELU, bias=bias_ap[:, o])


def _preds(nc, psum_pool, stage_pool, pwc, pwb, pbc, pbb,
           cls_tower, box_tower, out_d, H, W, R, pix_base, tag):
    """cls (20ch) + box/ctr (5ch) 3x3 prediction convs, 4-way column-tiled."""
    for it in range(H // R):
        _preds_tile(nc, psum_pool, stage_pool, pwc, pwb, pbc, pbb,
                    cls_tower, box_tower, out_d, H, W, R, pix_base, tag, it)


def _preds_tile(nc, psum_pool, stage_pool, pwc, pwb, pbc, pbb,
                cls_tower, box_tower, out_d, H, W, R, pix_base, tag, it):
    if True:
        r0 = it * R
        ps = psum_pool.tile([P, R, W], F32, tag="ps", name=f"pp_{tag}_{it}")
        specs = [
            (0, pwc, 0, cls_tower, 20),
            (32, pwc, 1, cls_tower, 20),
            (64, pwb, 0, box_tower, 5),
            (96, pwb, 1, box_tower, 5),
        ]
        for t in range(9):
            ky, kx = t // 3, t % 3
            for col, pw, c, tower, nout in specs:
                rhs = tower[:, c, r0 + ky:r0 + ky + R, kx:kx + W]
                nc.tensor.matmul(ps[col:col + nout], pw[:, c, t, :], rhs,
                                 start=(t == 0), stop=(t == 8),
                                 tile_position=(0, col))
        st = stage_pool.tile([32, R * W], F32, tag="st", bufs=4,
                             name=f"st_{tag}_{it}")
        st2 = stage_pool.tile([32, R * W], F32, tag="st", bufs=4,
                              name=f"s2_{tag}_{it}")
        sta = stage_pool.tile([32, R * W], F32, tag="st", bufs=4,
                              name=f"sa_{tag}_{it}")
        stb = stage_pool.tile([32, R * W], F32, tag="st", bufs=4,
                              name=f"sb_{tag}_{it}")
        flat = ps.rearrange("p r w -> p (r w)")
        nc.scalar.activation(sta[0:20], flat[0:20], IDENT, bias=pbc[0:20])
        nc.scalar.activation(stb[0:5], flat[64:69], IDENT, bias=pbb[0:5])
        nc.vector.tensor_tensor(st[0:20], sta[0:20], flat[32:52], ADD)
        nc.vector.tensor_tensor(st2[0:5], stb[0:5], flat[96:101], ADD)
        c0 = pix_base + r0 * W
        nc.sync.dma_start(out_d[0:20, c0:c0 + R * W], st[0:20])
        nc.sync.dma_start(out_d[20:25, c0:c0 + R * W], st2[0:5])


def _build():
    nc = bacc.Bacc("TRN2", target_bir_lowering=False, debug=False,
                   num_devices=8)

    # p3/p4 features ship as host-computed Winograd D planes; p5 spatial.
    x0_d = nc.dram_tensor("x0", (NCH, P, 32, 4, 66), BF16,
                          kind="ExternalInput")
    x1_d = nc.dram_tensor("x1", (NCH, P, 16, 4, 34), BF16,
                          kind="ExternalInput")
    x2_d = nc.dram_tensor("x2", (NCH, P, 18, 18), BF16, kind="ExternalInput")
    tw_d = nc.dram_tensor("tw", (2, NL, P, NCH, NCH, 3, 4, P), BF16,
                          kind="ExternalInput")
    dw_d = nc.dram_tensor("dw", (2, NL, P, NCH, NCH, 9, P), BF16,
                          kind="ExternalInput")
    sb_d = nc.dram_tensor("sb", (2, NL, NCH, P, 1), F32, kind="ExternalInput")
    pwc_d = nc.dram_tensor("pwc", (P, NCH, 9, 20), BF16, kind="ExternalInput")
    pwb_d = nc.dram_tensor("pwb", (P, NCH, 9, 5), BF16, kind="ExternalInput")
    pbc_d = nc.dram_tensor("pbc", (20, 1), F32, kind="ExternalInput")
    pbb_d = nc.dram_tensor("pbb", (5, 1), F32, kind="ExternalInput")
    out_d = nc.dram_tensor("out", (25, NPIX_TOTAL), F32, kind="ExternalOutput")

    ND3 = NCH * 4 * 32 * 66       # 16896: p3 D-plane elems/partition
    ND4 = NCH * 4 * 16 * 34       # 4352
    NS3 = NCH * 66 * 66           # 8712: p3 padded spatial elems/partition
    NS4 = NCH * 34 * 34           # 2312
    NS5 = NCH * 18 * 18           # 648

    with tile.TileContext(nc) as tc:
        with (
            tc.tile_pool(name="resident", bufs=1) as res_pool,
            tc.tile_pool(name="psum", bufs=2, space="PSUM") as psum_pool,
            tc.tile_pool(name="stage", bufs=4) as stage_pool,
        ):
            # D-plane ping-pong buffers (p3-sized; p4 carves sub-regions)
            d0f = res_pool.tile([P, ND3], BF16, name="d0f")
            d1f = res_pool.tile([P, ND3], BF16, name="d1f")
            fd4 = res_pool.tile([P, ND4], BF16, name="fd4")   # p4 feat D
            # spatial buffers: rotating + cls tower + box tower, per level
            sA3 = res_pool.tile([P, NS3], BF16, name="sA3")
            sB3 = res_pool.tile([P, NS3], BF16, name="sB3")
            sC3 = res_pool.tile([P, NS3], BF16, name="sC3")
            sA4 = res_pool.tile([P, NS4], BF16, name="sA4")
            sB4 = res_pool.tile([P, NS4], BF16, name="sB4")
            sC4 = res_pool.tile([P, NS4], BF16, name="sC4")
            p50 = res_pool.tile([P, NS5], BF16, name="p50")
            p51 = res_pool.tile([P, NS5], BF16, name="p51")
            p52 = res_pool.tile([P, NS5], BF16, name="p52")

            sbias = res_pool.tile([P, 2, NL, NCH, 1], F32, name="sbias")
            pwc = res_pool.tile([P, NCH, 9, 20], BF16, name="pwc")
            pwb = res_pool.tile([P, NCH, 9, 5], BF16, name="pwb")
            pbc = res_pool.tile([32, 1], F32, name="pbc")
            pbb = res_pool.tile([32, 1], F32, name="pbb")
            # weight ping-pong: resident tiles (real dep tracking) with
            # one-step-ahead DMA prefetch. Pooled tiles gate their loads
            # on ALL matmuls emitted earlier, serializing layer boundaries.
            tw_t = [res_pool.tile([P, NCH, NCH, 3, 4, P], BF16, name="twA"),
                    res_pool.tile([P, NCH, NCH, 3, 4, P], BF16, name="twB")]
            dw_t = [res_pool.tile([P, NCH, NCH, 9, P], BF16, name="dwA"),
                    res_pool.tile([P, NCH, NCH, 9, P], BF16, name="dwB")]

            D3 = [_d_view(d0f, 0, 32, 66), _d_view(d1f, 0, 32, 66)]
            D4 = [_d_view(d0f, 0, 16, 34), _d_view(d1f, 0, 16, 34)]
            fD4 = _d_view(fd4, 0, 16, 34)
            vA3 = _pad_view(sA3, 0, 64, 64)
            vB3 = _pad_view(sB3, 0, 64, 64)
            vC3 = _pad_view(sC3, 0, 64, 64)
            vA4 = _pad_view(sA4, 0, 32, 32)
            vB4 = _pad_view(sB4, 0, 32, 32)
            vC4 = _pad_view(sC4, 0, 32, 32)
            v5 = [_pad_view(p50, 0, 16, 16), _pad_view(p51, 0, 16, 16),
                  _pad_view(p52, 0, 16, 16)]

            # --- startup DMAs (sync HWDGE queue, consumption order) -------
            tw00 = tw_t[0]
            nc.sync.dma_start(D3[0][:, 0, 0:4], x0_d[0, :, 0:4])
            nc.sync.dma_start(D3[0][:, 0, 4:8], x0_d[0, :, 4:8])
            for kx in range(3):
                nc.sync.dma_start(tw00[:, 0, 0, kx], tw_d[0, 0, :, 0, 0, kx])
            nc.sync.dma_start(D3[0][:, 1, 0:4], x0_d[1, :, 0:4])
            nc.sync.dma_start(D3[0][:, 1, 4:8], x0_d[1, :, 4:8])
            nc.sync.dma_start(tw00[:, 1, 0], tw_d[0, 0, :, 1, 0])
            nc.sync.dma_start(tw00[:, 0, 1], tw_d[0, 0, :, 0, 1])
            nc.sync.dma_start(tw00[:, 1, 1], tw_d[0, 0, :, 1, 1])
            for q0, q1 in ((8, 16), (16, 24), (24, 32)):
                for c in range(NCH):
                    nc.sync.dma_start(D3[0][:, c, q0:q1], x0_d[c, :, q0:q1])
            # consts + p4/p5 feats on gpsimd SWDGE: no contention with the
            # startup-critical HWDGE traffic
            nc.gpsimd.dma_start(
                sbias[:],
                sb_d[:].rearrange("s l a p o -> p (s l a o)")
                       .rearrange("p (s l a o) -> p s l a o",
                                  s=2, l=NL, a=NCH))
            nc.gpsimd.dma_start(pwc[:], pwc_d[:])
            nc.gpsimd.dma_start(pwb[:], pwb_d[:])
            nc.gpsimd.dma_start(pbc[:20], pbc_d[:])
            nc.gpsimd.dma_start(pbb[:5], pbb_d[:])
            for c in range(NCH):
                nc.gpsimd.dma_start(fD4[:, c], x1_d[c])
                nc.gpsimd.dma_start(v5[0][:, c], x2_d[c])
            for v, lv in ((vA3, 64), (vB3, 64), (vC3, 64),
                          (vA4, 32), (vB4, 32), (vC4, 32)):
                _zero_ring(nc, v, lv, lv)
            _zero_ring(nc, v5[1], 16, 16)
            _zero_ring(nc, v5[2], 16, 16)

            # --- stems ---------------------------------------------------
            # 16 layer-steps: pass a (p3) cls l0-3, box l0-3; pass b
            # (p4 winograd + p5 direct) likewise. Step i computes with
            # tw_t[i%2]/dw_t[i%2]; step i+1's weights prefetch during i.
            # D rotation per stream: l0 reads feat D, T->D[1]; l1 reads
            # D[1], T->D[0]; l2 reads D[0], T->D[1]; l3 reads D[1] ->
            # tower spatial. p3's box stream re-fetches feat D into D[0];
            # p4's feat D lands in the persistent fd4.
            lv3c = ('w', D3, D3[0], vA3, vB3, 32, 64, 8)
            lv3b = ('w', D3, D3[0], vA3, vC3, 32, 64, 8)
            lv4c = ('w', D4, fD4, vA4, vB4, 16, 32, 16)
            lv4b = ('w', D4, fD4, vA4, vC4, 16, 32, 16)
            steps = ([('a', 0, l, [lv3c]) for l in range(NL)]
                     + [('a', 1, l, [lv3b]) for l in range(NL)]
                     + [('b', 0, l, [lv4c, ('5',)]) for l in range(NL)]
                     + [('b', 1, l, [lv4b, ('5',)]) for l in range(NL)])

            for i, (pss, s, l, lvls) in enumerate(steps):
                if i + 1 < len(steps):
                    pn, sn, ln, lvn = steps[i + 1]
                    tn = tw_t[(i + 1) % 2]
                    for o in range(NCH):
                        nc.sync.dma_start(tn[:, :, o], tw_d[sn, ln, :, :, o])
                    if any(lv[0] == '5' for lv in lvn):
                        dn = dw_t[(i + 1) % 2]
                        nc.sync.dma_start(dn[:], dw_d[sn, ln])
                twt = tw_t[i % 2]
                for lv in lvls:
                    if lv[0] == 'w':
                        (_, Dp, feat, rot, tower, Q, W, RQ) = lv
                        srcD = feat if l == 0 else Dp[l % 2]
                        dstS = tower if l == NL - 1 else rot
                        xt = None if l == NL - 1 else Dp[(l + 1) % 2]
                        _wino_layer(nc, psum_pool, stage_pool, twt,
                                    srcD, dstS, sbias[:, s, l],
                                    Q, W, RQ, f"w{pss}{Q}_{s}{l}",
                                    xform_to=xt)
                    else:
                        chain = _CLS_CHAIN if s == 0 else _BOX_CHAIN
                        si, di = chain[l]
                        _conv_layer(nc, psum_pool, dw_t[i % 2], v5[si],
                                    v5[di], sbias[:, s, l], 16, 16, 16,
                                    f"p5{pss}_{s}{l}")
                if pss == 'a' and s == 0 and l == 2:
                    # refetch p3 feat D for the box stream; D3[0] frees
                    # when this l2 stops reading it, transfer hides in l3
                    for c in range(NCH):
                        nc.sync.dma_start(D3[0][:, c], x0_d[c])
                if pss == 'b':
                    # one p3 preds tile per pass-b step: its matmuls have
                    # long-ready towers and backfill the boundary gap
                    # between this step's transforms and the next layer
                    _preds_tile(nc, psum_pool, stage_pool, pwc, pwb,
                                pbc, pbb, vB3, vC3, out_d, 64, 64, 8, 0,
                                "a0", i - 2 * NL)
            _preds(nc, psum_pool, stage_pool, pwc, pwb, pbc, pbb,
                   v5[2], v5[0], out_d, 16, 16, 16, 5120, "b1")
            _preds(nc, psum_pool, stage_pool, pwc, pwb, pbc, pbb,
                   vB4, vC4, out_d, 32, 32, 16, 4096, "b0")

    nc.compile()
    return nc


# direct-conv buffer rotation for p5 (v0=feat, v1, v2)
_CLS_CHAIN = [(0, 1), (1, 2), (2, 1), (1, 2)]
_BOX_CHAIN = [(0, 1), (1, 0), (0, 1), (1, 0)]


def _pack_tw(wcls, wbox):
    # [s, l, co, ci, ky, kx] -> G over ky -> [s, l, cip, cic, coc, kx, j, cop]
    w = np.stack([wcls, wbox]).astype(np.float32)  # (2, NL, 256, 256, 3, 3)
    w0, w1, w2 = w[..., 0, :], w[..., 1, :], w[..., 2, :]
    t = np.stack([w0, (w0 + w1 + w2) * 0.5, (w0 - w1 + w2) * 0.5, w2],
                 axis=-1)                          # (2, NL, 256, 256, 3, 4)
    t = t.reshape(2, NL, NCH, P, NCH, P, 3, 4)
    t = t.transpose(0, 1, 5, 4, 2, 6, 7, 3)        # s,l,cip,cic,coc,kx,j,cop
    return np.ascontiguousarray(t).astype(ml_dtypes.bfloat16)


def _pack_dw(wcls, wbox):
    # [s, l, co, ci, ky, kx] -> [s, l, cip, cic, coc, tap, cop]
    w = np.stack([wcls, wbox]).reshape(2, NL, NCH, P, NCH, P, 3, 3)
    w = w.transpose(0, 1, 5, 4, 2, 6, 7, 3)
    return np.ascontiguousarray(w.reshape(2, NL, P, NCH, NCH, 9, P)).astype(
        ml_dtypes.bfloat16)


def _pack_pred_w(w):
    # [co, ci, ky, kx] -> [cip, cic, tap, co]
    n = w.shape[0]
    w = w.reshape(n, NCH, P, 3, 3).transpose(2, 1, 3, 4, 0)
    return np.ascontiguousarray(w.reshape(P, NCH, 9, n)).astype(
        ml_dtypes.bfloat16)


def _feat_d(x, H, W):
    # (256, H, W) fp32 -> D planes (NCH, P, H/2, 4, W+2) bf16
    xp = np.pad(x.reshape(NCH, P, H, W), ((0, 0), (0, 0), (1, 1), (1, 1)))
    d0 = xp[:, :, 0:H:2, :]
    d1 = xp[:, :, 1:H + 1:2, :]
    d2 = xp[:, :, 2:H + 2:2, :]
    d3 = xp[:, :, 3:H + 2:2, :]
    D = np.stack([d0 - d2, d1 + d2, d2 - d1, d1 - d3], axis=3)
    return np.ascontiguousarray(D).astype(ml_dtypes.bfloat16)


def kernel(p3, p4, p5, stem_cls_w, stem_cls_b, stem_box_w, stem_box_b,
           pred_cls_w, pred_cls_b, pred_box_w, pred_box_b,
           pred_ctr_w, pred_ctr_b):
    if 'nc' not in _cached:
        _cached['nc'] = _build()
    nc = _cached['nc']

    B = p3.shape[0]
    scw, sbw = np.asarray(stem_cls_w), np.asarray(stem_box_w)
    tw = _pack_tw(scw, sbw)
    dw = _pack_dw(scw, sbw)
    sb = np.ascontiguousarray(
        np.stack([stem_cls_b, stem_box_b]).reshape(2, NL, NCH, P, 1),
        dtype=np.float32)
    pwc = _pack_pred_w(np.asarray(pred_cls_w))
    pwb = _pack_pred_w(np.concatenate([pred_box_w, pred_ctr_w], axis=0))
    pbc = np.asarray(pred_cls_b, np.float32).reshape(20, 1)
    pbb = np.concatenate([pred_box_b, pred_ctr_b]).astype(np.float32).reshape(5, 1)

    shared = {"tw": tw, "dw": dw, "sb": sb, "pwc": pwc, "pwb": pwb,
              "pbc": pbc, "pbb": pbb}
    xs = [np.asarray(p3, np.float32), np.asarray(p4, np.float32),
          np.asarray(p5, np.float32)]
    in_maps = []
    for b in range(B):
        m = dict(shared)
        m["x0"] = _feat_d(xs[0][b], 64, 64)
        m["x1"] = _feat_d(xs[1][b], 32, 32)
        m["x2"] = np.pad(xs[2][b].reshape(NCH, P, 16, 16),
                         ((0, 0), (0, 0), (1, 1), (1, 1))).astype(
                             ml_dtypes.bfloat16)
        in_maps.append(m)

    res = run_bass_kernel_spmd(nc, in_maps, core_ids=list(range(B)),
                               **_run_opts)
    _last['res'] = res
    out = np.stack([r["out"].T for r in res.results])
    return np.ascontiguousarray(out, dtype=np.float32)
